# revision 2
# baseline (speedup 1.0000x reference)
"""Trainium2 Bass kernel for the butterfly-CNN problem (nn_CNNLayer_30296699306356).

Network: input conv (k=2,s=2, 1->8 ch) + 10 butterfly conv levels (k=2,s=2,
channels double each level, relu, zero biases) + a per-block dense matmul
(1024 blocks of [8,2]) at the end.

v2 strategy (memory-regime; bf16 weight traffic is the roofline):
  - Levels 8, 9, 10 are ALL output-channel sharded across the 8 cores
    (1 + 4 + 16 MiB of bf16 weights per core instead of 8 + 4 + 16 with a
    replicated level 8). Two tiny AllGathers (32 KiB/rank each) rebuild the
    full activations after levels 8 and 9. Per-core HBM weight traffic:
    ~24.4 MiB vs ~33 MiB for the v1 kernel.
  - All streamed weights (w5..w10 shards) are packed host-side into ONE
    per-core DRAM tensor of 12 x 2 MiB chunks, in exact consumption order,
    and DMAed on the sync (qSPDynamicHW) ring through a single 8-deep tile
    pool. One FIFO in consumption order = no head-of-line blocking, no
    bandwidth sharing between competing weight streams, ~380 GB/s chunks.
  - Everything small / latency-critical (const loads, collective bounce
    copies, gather loads, final output) rides the scalar (qActDynamicHW)
    ring, so it never queues behind megabytes of weight traffic. In the v1
    kernel the post-AllGather x9 loads sat in the same FIFO as the w10
    stream, which both delayed the collective trigger and parked the weight
    DMA for ~30 us mid-kernel.
  - Levels 5..10 run in bf16 (fp32 PSUM accum); levels in..4 stay fp32
    im2col-packed, replicated. Measured rel err vs fp32 reference: ~5.5e-3.

kernel(**inputs) takes the FULL unsharded inputs and returns the FULL output.
"""

import ml_dtypes
import numpy as np

NCORES = 8
B = 16
P = 128
C = 8
NLVL = 10
BF16 = ml_dtypes.bfloat16

_CACHE = {}


# ---------------------------------------------------------------- host prep

def _host_prep(inputs):
    """Build the per-core input maps (numpy only)."""
    ind = np.ascontiguousarray(np.asarray(inputs["in_data"], np.float32))
    f = {l: np.asarray(inputs[f"f{l}"], np.float32) for l in range(1, NLVL + 1)}
    f0 = np.asarray(inputs["in_filter"], np.float32)     # [2, 1, 8]
    fd = np.asarray(inputs["fea_dense"], np.float32)     # [1024, 8, 2]

    shared = {}
    # r0 [32, 64, 16]: r0[row, wHi, b] = in[b, wHi*32 + row]
    shared["r0"] = np.ascontiguousarray(
        ind[:, :, 0].reshape(B, 64, 32).transpose(2, 1, 0))

    # w0 [32, 128]: rows (2*wsub + k), cols (wsub*8 + co)
    w0 = np.zeros((32, 128), np.float32)
    for wsub in range(16):
        for k in range(2):
            w0[2 * wsub + k, wsub * 8:wsub * 8 + 8] = f0[k, 0, :]
    shared["w0"] = w0

    # packed levels 1..4 stacked: wpk [4, 128, 128]
    wpk = np.zeros((4, 128, 128), np.float32)
    for lvl in range(1, 5):
        cin = 2 ** (lvl - 1) * C
        cout = 2 ** lvl * C
        s_out = (128 // cin) // 2
        for wso in range(s_out):
            for k in range(2):
                wsi = 2 * wso + k
                wpk[lvl - 1, wsi * cin:(wsi + 1) * cin,
                    wso * cout:(wso + 1) * cout] = f[lvl][k]
    shared["wpk"] = wpk

    # kt-major packs for the streamed levels
    w5p = f[5].astype(BF16).reshape(2, 1, 128, 256).transpose(2, 0, 1, 3).reshape(128, 512)
    w6p = f[6].astype(BF16).reshape(2, 2, 128, 512).transpose(2, 0, 1, 3).reshape(128, 2048)
    w7p = f[7].astype(BF16).reshape(2, 4, 128, 1024).transpose(2, 0, 1, 3).reshape(128, 8192)
    f8b = f[8].astype(BF16)
    f9b = f[9].astype(BF16)
    f10b = f[10].astype(BF16)

    in_maps = []
    for r in range(NCORES):
        # w8 output shard (256 couts): [128, kt=16, 256] flattened
        w8s = np.ascontiguousarray(
            f8b[:, :, r * 256:(r + 1) * 256]
            .reshape(2, 8, 128, 256).transpose(2, 0, 1, 3).reshape(128, 4096))
        chunk0 = np.concatenate(
            [w5p, w6p, w8s, np.zeros((128, 1536), BF16)], axis=1)

        # w9 output shard (512 couts): chunks [128, (q j c)] per k
        blk9 = f9b[:, :, r * 512:(r + 1) * 512]
        v9 = blk9.reshape(2, 4, 4, 128, 512).transpose(0, 1, 3, 2, 4)
        ck2 = v9[0].transpose(1, 0, 2, 3).reshape(128, 8192)
        ck3 = v9[1].transpose(1, 0, 2, 3).reshape(128, 8192)

        # w10 output shard (1024 couts): 16 units m=(k*8+q) of [128, (j c)]
        blk10 = f10b[:, :, r * 1024:(r + 1) * 1024]
        v10 = blk10.reshape(2, 8, 4, 128, 1024).transpose(0, 1, 3, 2, 4)
        units = [v10[mm // 8, mm % 8].reshape(128, 4096) for mm in range(16)]
        w10cks = [np.concatenate([units[2 * i], units[2 * i + 1]], axis=1)
                  for i in range(8)]

        ws = np.ascontiguousarray(np.stack(
            [chunk0, w7p, np.ascontiguousarray(ck2), np.ascontiguousarray(ck3)]
            + w10cks))  # [12, 128, 8192] bf16

        # fea_dense shard, per-o flattened, tiled over the 16 batch partitions
        blkfd = fd[r * 128:(r + 1) * 128]                  # [128, 8, 2]
        flat = blkfd.transpose(2, 0, 1).reshape(2, 1024)   # [o, 1024]
        fdt = np.ascontiguousarray(np.broadcast_to(flat[None], (B, 2, 1024)))

        m = dict(shared)
        m["ws"] = ws
        m["fdt"] = fdt
        in_maps.append(m)
    return in_maps


# ---------------------------------------------------------------- bass build

def _build():
    import concourse.bass as bass
    import concourse.mybir as mybir
    import concourse.tile as tile
    from concourse import bacc

    f32 = mybir.dt.float32
    bf16 = mybir.dt.bfloat16
    RELU = mybir.ActivationFunctionType.Relu

    nc = bacc.Bacc("TRN2", target_bir_lowering=False, debug=False,
                   num_devices=NCORES)

    def inp(name, shape, dt=f32):
        return nc.dram_tensor(name, shape, dt, kind="ExternalInput").ap()

    r0 = inp("r0", [32, 64, 16])
    w0 = inp("w0", [32, 128])
    wpk = inp("wpk", [4, 128, 128])
    ws = inp("ws", [12, 128, 8192], bf16)
    fdt = inp("fdt", [B, 2, 1024])
    out = nc.dram_tensor("out", [B, 128, 2], f32, kind="ExternalOutput").ap()

    with tile.TileContext(nc) as tc:
        with (
            tc.tile_pool(name="const", bufs=1) as constp,
            tc.tile_pool(name="actp", bufs=3) as actp,
            tc.tile_pool(name="wsp", bufs=8) as wsp,
            tc.tile_pool(name="bigp", bufs=1) as bigp,
            tc.tile_pool(name="psA", bufs=2, space="PSUM") as psA,
            tc.tile_pool(name="psB", bufs=4, space="PSUM") as psB,
            tc.tile_pool(name="psC", bufs=2, space="PSUM") as psC,
            tc.tile_pool(name="dramp", bufs=1, space="DRAM") as dramp,
        ):
            # ---- const loads on the scalar (ACT) ring
            r0sb = constp.tile([32, 64, 16], f32, name="r0sb")
            nc.scalar.dma_start(r0sb[:], r0)
            w0sb = constp.tile([32, 128], f32, name="w0sb")
            nc.scalar.dma_start(w0sb[:], w0)
            wpksb = constp.tile([128, 4, 128], f32, name="wpksb")
            nc.scalar.dma_start(wpksb[:], wpk.rearrange("l p c -> p l c"))
            fdsb = constp.tile([B, 2, 1024], f32, name="fdsb")
            nc.scalar.dma_start(fdsb[:], fdt)

            # ---- the weight stream: 12 x 2 MiB chunks on the sync ring
            ck = []
            for i in range(12):
                t = wsp.tile([128, 8192], bf16, name=f"ck{i}", tag="ws")
                nc.sync.dma_start(t[:], ws[i])
                ck.append(t)

            # ---- input conv + packed levels 1..4 (all [128, 64, 16])
            xprev = None
            for lvl in range(5):
                xn = actp.tile([128, 64, 16], bf16 if lvl == 4 else f32,
                               name=f"x{lvl}", tag="xl")
                for ch in range(2):
                    ps = psA.tile([128, 32, 16], f32, name="psA", tag="psA")
                    if lvl == 0:
                        nc.tensor.matmul(
                            ps[:], w0sb[:], r0sb[:, ch * 32:(ch + 1) * 32, :],
                            start=True, stop=True)
                    else:
                        nc.tensor.matmul(
                            ps[:], wpksb[:, lvl - 1, :],
                            xprev[:, ch * 32:(ch + 1) * 32, :],
                            start=True, stop=True)
                    nc.scalar.activation(
                        xn[:, ch * 32:(ch + 1) * 32, :], ps[:], RELU)
                xprev = xn

            # ---- standard levels 5..7 (weights stationary)
            def std_level(xin, wsb, cin_t, cout_t, w_out, name):
                # xin [128, cin_t, 2*w_out, 16]; wsb [128, 2*cin_t, co]
                xn = actp.tile([128, cout_t, w_out, 16], bf16,
                               name=name, tag="xl")
                for ct in range(cout_t):
                    ps = psA.tile([128, w_out, 16], f32, name="psA", tag="psA")
                    for cit in range(cin_t):
                        rhs2 = xin[:, cit].rearrange(
                            "p (w two) b -> p two w b", two=2)
                        for k in range(2):
                            nc.tensor.matmul(
                                ps[:],
                                wsb[:, k * cin_t + cit,
                                    ct * 128:(ct + 1) * 128],
                                rhs2[:, k],
                                start=(cit == 0 and k == 0),
                                stop=(cit == cin_t - 1 and k == 1))
                    nc.scalar.activation(xn[:, ct], ps[:], RELU)
                return xn

            w5v = ck[0][:, 0:512].rearrange("p (t c) -> p t c", c=256)
            w6v = ck[0][:, 512:2560].rearrange("p (t c) -> p t c", c=512)
            w7v = ck[1][:].rearrange("p (t c) -> p t c", c=1024)
            x5 = std_level(xprev[:, None], w5v, 1, 2, 32, "x5")
            x6 = std_level(x5, w6v, 2, 4, 16, "x6")
            x7 = std_level(x6, w7v, 4, 8, 8, "x7")

            # ---- level 8, output-sharded (256 couts = 2 tiles of 128)
            w8v = ck[0][:, 2560:6656].rearrange("p (t c) -> p t c", c=256)
            x8loc = bigp.tile([128, 2, 4, 16], bf16, name="x8loc")
            for ctl in range(2):
                ps = psA.tile([128, 4, 16], f32, name="psA", tag="psA")
                for cit in range(8):
                    rhs2 = x7[:, cit].rearrange(
                        "p (w two) b -> p two w b", two=2)
                    for k in range(2):
                        nc.tensor.matmul(
                            ps[:],
                            w8v[:, k * 8 + cit, ctl * 128:(ctl + 1) * 128],
                            rhs2[:, k],
                            start=(cit == 0 and k == 0),
                            stop=(cit == 7 and k == 1))
                nc.scalar.activation(x8loc[:, ctl], ps[:], RELU)

            # ---- AllGather x8 -> x8g [128, 8, 2, 4, 16]
            ag8_in = dramp.tile([1, 128, 2, 4, 16], bf16, name="ag8_in")
            ag8_out = dramp.tile([NCORES, 128, 2, 4, 16], bf16, name="ag8_out",
                                 addr_space="Shared")
            nc.scalar.dma_start(ag8_in[0], x8loc[:])
            nc.gpsimd.collective_compute(
                "AllGather", mybir.AluOpType.bypass,
                replica_groups=[list(range(NCORES))],
                ins=[ag8_in.opt()], outs=[ag8_out.opt()])
            x8g = bigp.tile([128, NCORES, 2, 4, 16], bf16, name="x8g")
            nc.scalar.dma_start(
                x8g[:], ag8_out[:].rearrange("r p l w b -> p r l w b"))

            # ---- level 9 (512-cout shard, 4 accumulators, chunks 2..3)
            ps9 = [psB.tile([128, 2, 16], f32, name=f"ps9_{ct}", tag="psB")
                   for ct in range(4)]
            for m in range(8):
                k, q = divmod(m, 4)
                for j in range(4):
                    cit = q * 4 + j
                    s, lc = divmod(cit, 2)
                    rhs = x8g[:, s, lc].rearrange(
                        "p (w two) b -> p two w b", two=2)[:, k]
                    base = (m % 4) * 2048 + j * 512
                    for ct in range(4):
                        nc.tensor.matmul(
                            ps9[ct][:],
                            ck[2 + m // 4][:, base + ct * 128:base + (ct + 1) * 128],
                            rhs,
                            start=(m == 0 and j == 0),
                            stop=(m == 7 and j == 3))
            x9loc = bigp.tile([128, 4, 2, 16], bf16, name="x9loc")
            for ct in range(4):
                nc.scalar.activation(x9loc[:, ct], ps9[ct][:], RELU)

            # ---- AllGather x9 -> x9g [128, 8, 4, 2, 16]
            ag9_in = dramp.tile([1, 128, 4, 2, 16], bf16, name="ag9_in")
            ag9_out = dramp.tile([NCORES, 128, 4, 2, 16], bf16, name="ag9_out",
                                 addr_space="Shared")
            nc.scalar.dma_start(ag9_in[0], x9loc[:])
            nc.gpsimd.collective_compute(
                "AllGather", mybir.AluOpType.bypass,
                replica_groups=[list(range(NCORES))],
                ins=[ag9_in.opt()], outs=[ag9_out.opt()])
            x9g = bigp.tile([128, NCORES, 4, 2, 16], bf16, name="x9g")
            nc.scalar.dma_start(
                x9g[:], ag9_out[:].rearrange("r p t k b -> p r t k b"))

            # ---- level 10 (1024-cout shard, acts stationary, chunks 4..11)
            ps10 = [psC.tile([B, 512], f32, name=f"ps10_{cb}", tag="psC")
                    for cb in range(2)]
            for i in range(16):
                k, q = divmod(i, 8)
                base_u = (i % 2) * 4096
                for j in range(4):
                    lhsT = x9g[:, q, j, k, :]
                    for cb in range(2):
                        nc.tensor.matmul(
                            ps10[cb][:], lhsT,
                            ck[4 + i // 2][:, base_u + j * 1024 + cb * 512:
                                           base_u + j * 1024 + (cb + 1) * 512],
                            start=(i == 0 and j == 0),
                            stop=(i == 15 and j == 3))
            x10 = bigp.tile([B, 1024], f32, name="x10")
            for cb in range(2):
                nc.scalar.activation(
                    x10[:, cb * 512:(cb + 1) * 512], ps10[cb][:], RELU)

            # ---- final per-block einsum on the vector engine
            osb = bigp.tile([B, 128, 2], f32, name="osb")
            for o in range(2):
                prod = bigp.tile([B, 1024], f32, name=f"prod{o}")
                nc.vector.tensor_tensor(
                    prod[:], x10[:], fdsb[:, o, :], mybir.AluOpType.mult)
                nc.vector.tensor_reduce(
                    osb[:, :, o],
                    prod.rearrange("p (k c) -> p k c", c=8),
                    mybir.AxisListType.X, mybir.AluOpType.add)
            nc.scalar.dma_start(out, osb[:])

    nc.compile()
    return nc


# ------------------------------------------------------------------- kernel

def kernel(**inputs):
    from concourse.bass_utils import run_bass_kernel_spmd

    in_maps = _host_prep(inputs)
    if "nc" not in _CACHE:
        _CACHE["nc"] = _build()
    nc = _CACHE["nc"]
    res = run_bass_kernel_spmd(nc, in_maps, core_ids=list(range(NCORES)))
    parts = [res.results[r]["out"] for r in range(NCORES)]  # each [16, 128, 2]
    full = np.concatenate(parts, axis=1)                    # [16, 1024, 2]
    return np.ascontiguousarray(full.reshape(B, 2048, 1).astype(np.float32))


# revision 8
# speedup vs baseline: 1.1307x; 1.1307x over previous
"""Trainium2 Bass kernel for the butterfly-CNN problem (nn_CNNLayer_30296699306356).

Network: input conv (k=2,s=2, 1->8 ch) + 10 butterfly conv levels (k=2,s=2,
channels double each level, relu, zero biases) + a per-block dense matmul
(1024 blocks of [8,2]) at the end.

v4 strategy (memory-regime; bf16 weight traffic is the roofline):
  - Levels 8, 9, 10 output-channel sharded across the 8 cores (1 + 4 + 16 MiB
    bf16 per core, ~24.4 MiB total HBM traffic vs ~33 replicated). Two tiny
    AllGathers (32 KiB/rank) rebuild the full activations after levels 8/9.
  - Measured law: ncfw collectives make almost no progress while all 8 cores
    stream weights at ~350 GB/s. So the kernel deliberately PAUSES the
    stream around each collective: the gather loads sit in the sync-ring
    FIFO between weight chunks; when they block on the AllGather semaphore,
    all 8 cores' streams go quiet simultaneously (SPMD symmetry), the
    collective completes at its uncontended floor, and the stream resumes.
    Bounce stores also ride the sync FIFO (the scalar ring is starved to
    ~4 KiB/packet-round under the storm; +11 us trigger delay measured).
  - One weight stream in exact consumption order (consts, w7, w8 shard,
    w9 shard, w10 shard as 12 x 2 MiB chunks) through a 9-deep pool.
  - Whole net in bf16 (fp32 PSUM): fp32 early levels cost 1.3 us/matmul
    (two-pass) on the serial chain, bf16 0.2 us. Early relus on the vector
    engine. ~40 warm-up matmuls during the ~10 us DMA prologue dead-zone
    release the PE HAM clock gate (1.2 -> 2.4 GHz) before the real chain.
  - PSUM accumulators each get a full (bank-aligned) pool buffer: matmul
    dst slices at sub-bank offsets silently corrupt accumulation (measured).

kernel(**inputs) takes the FULL unsharded inputs and returns the FULL output.
"""

import ml_dtypes
import numpy as np

NCORES = 8
B = 16
P = 128
C = 8
NLVL = 10
BF16 = ml_dtypes.bfloat16

# stream column offsets (bf16 elems per partition; chunks of 8192)
OFF_W7 = 0
OFF_W8 = 8192
OFF_W9 = 12288          # + 2048*m
OFF_W10 = 28672         # + 4096*u

_CACHE = {}


# ---------------------------------------------------------------- host prep

def _host_prep(inputs):
    """Build the per-core input maps (numpy only)."""
    ind = np.ascontiguousarray(np.asarray(inputs["in_data"], np.float32))
    f = {l: np.asarray(inputs[f"f{l}"], np.float32) for l in range(1, NLVL + 1)}
    f0 = np.asarray(inputs["in_filter"], np.float32)     # [2, 1, 8]
    fd = np.asarray(inputs["fea_dense"], np.float32)     # [1024, 8, 2]

    shared = {}
    # c32 [32, 1152] bf16: r0 (1024) | w0 (128)
    r0 = ind[:, :, 0].reshape(B, 64, 32).transpose(2, 1, 0)   # [32, 64, 16]
    w0 = np.zeros((32, 128), np.float32)
    for wsub in range(16):
        for k in range(2):
            w0[2 * wsub + k, wsub * 8:wsub * 8 + 8] = f0[k, 0, :]
    shared["c32"] = np.ascontiguousarray(np.concatenate(
        [r0.reshape(32, 1024), w0], axis=1).astype(BF16))

    # c128 [128, 3072] bf16: wpk (512) | w5 (512) | w6 (2048)
    wpk = np.zeros((4, 128, 128), np.float32)
    for lvl in range(1, 5):
        cin = 2 ** (lvl - 1) * C
        cout = 2 ** lvl * C
        s_out = (128 // cin) // 2
        for wso in range(s_out):
            for k in range(2):
                wsi = 2 * wso + k
                wpk[lvl - 1, wsi * cin:(wsi + 1) * cin,
                    wso * cout:(wso + 1) * cout] = f[lvl][k]
    w5p = f[5].astype(BF16).reshape(2, 1, 128, 256).transpose(2, 0, 1, 3).reshape(128, 512)
    w6p = f[6].astype(BF16).reshape(2, 2, 128, 512).transpose(2, 0, 1, 3).reshape(128, 2048)
    shared["c128"] = np.ascontiguousarray(np.concatenate(
        [wpk.transpose(1, 0, 2).reshape(128, 512).astype(BF16), w5p, w6p], axis=1))

    w7p = f[7].astype(BF16).reshape(2, 4, 128, 1024).transpose(2, 0, 1, 3).reshape(128, 8192)
    f8b = f[8].astype(BF16)
    f9b = f[9].astype(BF16)
    f10b = f[10].astype(BF16)

    in_maps = []
    for r in range(NCORES):
        # w8 output shard (256 couts): [128, kt=16, 256] flattened
        w8s = (f8b[:, :, r * 256:(r + 1) * 256]
               .reshape(2, 8, 128, 256).transpose(2, 0, 1, 3).reshape(128, 4096))
        # w9 output shard (512 couts): units m = k*4+q of [128, (j c)]
        blk9 = f9b[:, :, r * 512:(r + 1) * 512]
        v9 = blk9.reshape(2, 4, 4, 128, 512).transpose(0, 1, 3, 2, 4)
        w9u = [v9[mm // 4, mm % 4].reshape(128, 2048) for mm in range(8)]
        # w10 output shard (1024 couts): units u = k*8+q of [128, (j c)]
        blk10 = f10b[:, :, r * 1024:(r + 1) * 1024]
        v10 = blk10.reshape(2, 8, 4, 128, 1024).transpose(0, 1, 3, 2, 4)
        w10u = [v10[u // 8, u % 8].reshape(128, 4096) for u in range(16)]

        flat = np.concatenate(
            [w7p, w8s] + w9u + w10u + [np.zeros((128, 4096), BF16)], axis=1)
        ws = np.ascontiguousarray(
            flat.reshape(128, 12, 8192).transpose(1, 0, 2))  # [12, 128, 8192]

        blkfd = fd[r * 128:(r + 1) * 128]
        fdt = np.ascontiguousarray(np.broadcast_to(
            blkfd.transpose(2, 0, 1).reshape(2, 1024)[None], (B, 2, 1024)))

        m = dict(shared)
        m["ws"] = ws
        m["fdt"] = fdt
        in_maps.append(m)
    return in_maps


# ---------------------------------------------------------------- bass build

def _build():
    import concourse.bass as bass
    import concourse.mybir as mybir
    import concourse.tile as tile
    from concourse import bacc

    f32 = mybir.dt.float32
    bf16 = mybir.dt.bfloat16
    RELU = mybir.ActivationFunctionType.Relu

    nc = bacc.Bacc("TRN2", target_bir_lowering=False, debug=False,
                   num_devices=NCORES)

    def inp(name, shape, dt):
        return nc.dram_tensor(name, shape, dt, kind="ExternalInput").ap()

    c32 = inp("c32", [32, 1152], bf16)
    c128 = inp("c128", [128, 3072], bf16)
    ws = inp("ws", [12, 128, 8192], bf16)
    fdt = inp("fdt", [B, 2, 1024], f32)
    out = nc.dram_tensor("out", [B, 128, 2], f32, kind="ExternalOutput").ap()

    def chunk_slice(ck, col, n):
        c, off = divmod(col, 8192)
        assert off + n <= 8192, (col, n)
        return ck[c][:, off:off + n]

    with tile.TileContext(nc) as tc:
        with (
            tc.tile_pool(name="const", bufs=1) as constp,
            tc.tile_pool(name="actp", bufs=3) as actp,
            tc.tile_pool(name="wsp", bufs=9) as wsp,
            tc.tile_pool(name="bigp", bufs=1) as bigp,
            tc.tile_pool(name="psA", bufs=2, space="PSUM") as psA,
            tc.tile_pool(name="psB", bufs=4, space="PSUM") as psB,
            tc.tile_pool(name="psC", bufs=2, space="PSUM") as psC,
            tc.tile_pool(name="dramp", bufs=1, space="DRAM") as dramp,
        ):
            # ---- dram tiles for the collectives
            ag8_in = dramp.tile([1, 128, 2, 4, 16], bf16, name="ag8_in")
            ag8_out = dramp.tile([NCORES, 128, 2, 4, 16], bf16, name="ag8_out",
                                 addr_space="Shared")
            ag9_in = dramp.tile([1, 128, 4, 2, 16], bf16, name="ag9_in")
            ag9_out = dramp.tile([NCORES, 128, 4, 2, 16], bf16, name="ag9_out",
                                 addr_space="Shared")

            # ---- PE warm-up: ~40 junk matmuls in the DMA-prologue dead zone
            # release the HAM clock gate before the real serial chain starts.
            wrm = bigp.tile([128, 512], bf16, name="wrm")
            nc.vector.memset(wrm[:], 0)
            wps = psC.tile([B, 512], f32, name="wps", tag="psC")
            for _ in range(40):
                nc.tensor.matmul(wps[:], wrm[:, 0:16], wrm[:],
                                 start=True, stop=True)

            # ---- sync ring FIFO: consts, then the weight stream with the
            # collective bounce stores + (pausing) gather loads interleaved.
            c32sb = constp.tile([32, 1152], bf16, name="c32sb")
            nc.sync.dma_start(c32sb[:], c32)
            c128sb = constp.tile([128, 3072], bf16, name="c128sb")
            nc.sync.dma_start(c128sb[:], c128)

            x8loc = bigp.tile([128, 2, 4, 16], bf16, name="x8loc")
            x8g = bigp.tile([128, NCORES, 2, 4, 16], bf16, name="x8g")
            x9loc = bigp.tile([128, 4, 2, 16], bf16, name="x9loc")
            x9g = bigp.tile([128, NCORES, 4, 2, 16], bf16, name="x9g")

            ck = []

            def stream_chunks(upto):
                while len(ck) < upto:
                    i = len(ck)
                    t = wsp.tile([128, 8192], bf16, name=f"ck{i}", tag="ws")
                    nc.sync.dma_start(t[:], ws[i])
                    ck.append(t)

            stream_chunks(3)   # w7, w8s, w9 m0..m5 in flight

            # ---- scalar ring: only ends-of-kernel traffic
            fdsb = constp.tile([B, 2, 1024], f32, name="fdsb")
            nc.scalar.dma_start(fdsb[:], fdt)

            r0v = c32sb[:, 0:1024].rearrange("p (w b) -> p w b", b=16)
            w0v = c32sb[:, 1024:1152]
            wpkv = c128sb[:, 0:512].rearrange("p (l c) -> p l c", c=128)

            # ---- input conv + packed levels 1..4 (bf16, vector relu)
            xprev = None
            for lvl in range(5):
                xn = actp.tile([128, 64, 16], bf16, name=f"x{lvl}", tag="xl")
                for ch in range(2):
                    ps = psA.tile([128, 32, 16], f32, name="psA", tag="psA")
                    if lvl == 0:
                        nc.tensor.matmul(
                            ps[:], w0v, r0v[:, ch * 32:(ch + 1) * 32, :],
                            start=True, stop=True)
                    else:
                        nc.tensor.matmul(
                            ps[:], wpkv[:, lvl - 1],
                            xprev[:, ch * 32:(ch + 1) * 32, :],
                            start=True, stop=True)
                    nc.vector.tensor_scalar_max(
                        xn[:, ch * 32:(ch + 1) * 32, :], ps[:], 0.0)
                xprev = xn

            # ---- standard levels 5..7 (weights stationary, vector relu)
            def std_level(xin, wsb, cin_t, cout_t, w_out, name):
                xn = actp.tile([128, cout_t, w_out, 16], bf16,
                               name=name, tag="xl")
                for ct in range(cout_t):
                    ps = psA.tile([128, w_out, 16], f32, name="psA", tag="psA")
                    for cit in range(cin_t):
                        rhs2 = xin[:, cit].rearrange(
                            "p (w two) b -> p two w b", two=2)
                        for k in range(2):
                            nc.tensor.matmul(
                                ps[:],
                                wsb[:, k * cin_t + cit,
                                    ct * 128:(ct + 1) * 128],
                                rhs2[:, k],
                                start=(cit == 0 and k == 0),
                                stop=(cit == cin_t - 1 and k == 1))
                    nc.vector.tensor_scalar_max(xn[:, ct], ps[:], 0.0)
                return xn

            w5v = c128sb[:, 512:1024].rearrange("p (t c) -> p t c", c=256)
            w6v = c128sb[:, 1024:3072].rearrange("p (t c) -> p t c", c=512)
            w7v = chunk_slice(ck, OFF_W7, 8192).rearrange(
                "p (t c) -> p t c", c=1024)
            x5 = std_level(xprev[:, None], w5v, 1, 2, 32, "x5")
            x6 = std_level(x5, w6v, 2, 4, 16, "x6")
            x7 = std_level(x6, w7v, 4, 8, 8, "x7")

            # ---- level 8, output-sharded (256 couts = 2 tiles of 128)
            w8v = chunk_slice(ck, OFF_W8, 4096).rearrange(
                "p (t c) -> p t c", c=256)
            for ctl in range(2):
                ps = psA.tile([128, 4, 16], f32, name="psA", tag="psA")
                for cit in range(8):
                    rhs2 = x7[:, cit].rearrange(
                        "p (w two) b -> p two w b", two=2)
                    for k in range(2):
                        nc.tensor.matmul(
                            ps[:],
                            w8v[:, k * 8 + cit, ctl * 128:(ctl + 1) * 128],
                            rhs2[:, k],
                            start=(cit == 0 and k == 0),
                            stop=(cit == 7 and k == 1))
                nc.scalar.activation(x8loc[:, ctl], ps[:], RELU)

            # ---- AllGather x8. Bounce store + gather load ride the sync
            # FIFO: the load's semaphore wait IS the stream pause (all 8
            # cores go quiet together, the collective runs uncontended).
            nc.sync.dma_start(ag8_in[0], x8loc[:])
            nc.gpsimd.collective_compute(
                "AllGather", mybir.AluOpType.bypass,
                replica_groups=[list(range(NCORES))],
                ins=[ag8_in.opt()], outs=[ag8_out.opt()])
            stream_chunks(4)   # one more chunk lands while AG8 starts
            nc.sync.dma_start(
                x8g[:], ag8_out[:].rearrange("r p l w b -> p r l w b"))
            stream_chunks(6)   # stream resumes behind the gather load

            # ---- level 9 (512-cout shard; 4 bank-aligned accumulators)
            ps9 = [psB.tile([128, 2, 16], f32, name=f"ps9_{ct}", tag="psB")
                   for ct in range(4)]
            for m in range(8):
                k, q = divmod(m, 4)
                for j in range(4):
                    cit = q * 4 + j
                    s, lc = divmod(cit, 2)
                    rhs = x8g[:, s, lc].rearrange(
                        "p (w two) b -> p two w b", two=2)[:, k]
                    for ct in range(4):
                        nc.tensor.matmul(
                            ps9[ct][:],
                            chunk_slice(ck, OFF_W9 + 2048 * m + 512 * j
                                        + 128 * ct, 128),
                            rhs,
                            start=(m == 0 and j == 0),
                            stop=(m == 7 and j == 3))
            for ct in range(4):
                nc.scalar.activation(x9loc[:, ct], ps9[ct][:], RELU)

            # ---- AllGather x9 (same pause pattern)
            nc.sync.dma_start(ag9_in[0], x9loc[:])
            nc.gpsimd.collective_compute(
                "AllGather", mybir.AluOpType.bypass,
                replica_groups=[list(range(NCORES))],
                ins=[ag9_in.opt()], outs=[ag9_out.opt()])
            stream_chunks(7)
            nc.sync.dma_start(
                x9g[:], ag9_out[:].rearrange("r p t k b -> p r t k b"))
            stream_chunks(12)

            # ---- level 10 (1024-cout shard, acts stationary)
            ps10 = [psC.tile([B, 512], f32, name=f"ps10_{cb}", tag="psC")
                    for cb in range(2)]
            for u in range(16):
                k, q = divmod(u, 8)
                for j in range(4):
                    lhsT = x9g[:, q, j, k, :]
                    for cb in range(2):
                        nc.tensor.matmul(
                            ps10[cb][:], lhsT,
                            chunk_slice(ck, OFF_W10 + 4096 * u + 1024 * j
                                        + 512 * cb, 512),
                            start=(u == 0 and j == 0),
                            stop=(u == 15 and j == 3))
            x10 = bigp.tile([B, 1024], f32, name="x10")
            for cb in range(2):
                nc.scalar.activation(
                    x10[:, cb * 512:(cb + 1) * 512], ps10[cb][:], RELU)

            # ---- final per-block einsum on the vector engine
            osb = bigp.tile([B, 128, 2], f32, name="osb")
            for o in range(2):
                prod = bigp.tile([B, 1024], f32, name=f"prod{o}")
                nc.vector.tensor_tensor(
                    prod[:], x10[:], fdsb[:, o, :], mybir.AluOpType.mult)
                nc.vector.tensor_reduce(
                    osb[:, :, o],
                    prod.rearrange("p (k c) -> p k c", c=8),
                    mybir.AxisListType.X, mybir.AluOpType.add)
            nc.scalar.dma_start(out, osb[:])

    nc.compile()
    return nc


# ------------------------------------------------------------------- kernel

def kernel(**inputs):
    from concourse.bass_utils import run_bass_kernel_spmd

    in_maps = _host_prep(inputs)
    if "nc" not in _CACHE:
        _CACHE["nc"] = _build()
    nc = _CACHE["nc"]
    res = run_bass_kernel_spmd(nc, in_maps, core_ids=list(range(NCORES)))
    parts = [res.results[r]["out"] for r in range(NCORES)]  # each [16, 128, 2]
    full = np.concatenate(parts, axis=1)                    # [16, 1024, 2]
    return np.ascontiguousarray(full.reshape(B, 2048, 1).astype(np.float32))


# revision 9
# speedup vs baseline: 1.2146x; 1.0741x over previous
"""Trainium2 Bass kernel for the butterfly-CNN problem (nn_CNNLayer_30296699306356).

Network: input conv (k=2,s=2, 1->8 ch) + 10 butterfly conv levels (k=2,s=2,
channels double each level, relu, zero biases) + a per-block dense matmul
(1024 blocks of [8,2]) at the end.

v5 strategy (memory-regime; bf16 weight traffic is the roofline):
  - Measured law on this system: the FIRST collective of a kernel cannot
    move data before a fixed ~84 us post-launch readiness point, no matter
    when it is triggered and even with all DMA quiet (ncfw/TOPSP startup).
    So the kernel uses exactly ONE collective and schedules everything
    around that floor: levels 8 and below are replicated (w8's extra 7 MiB
    of stream traffic is free - it rides during the floor window), level 9
    is output-sharded and triggers the single x9 AllGather at ~58 us, and
    the stream pauses briefly at ~85 us (the gather load sits in the sync
    FIFO) so the AllGather data phase runs uncontended right at the floor.
  - Level 10 is output-channel sharded (16 MiB/core); by AllGather-done all
    its weights are resident, so it runs pure-PE (~28 us) and the kernel
    ends at roughly floor + AllGather + L10 + tail.
  - One weight stream in exact consumption order (consts, w7, w8, w9 shard,
    w10 shard = 15 x 2 MiB chunks) through a 9-deep pool on the sync ring;
    scalar ring only carries fea_dense and the output (it is starved to
    ~4 KiB per packet-round while the stream runs).
  - Whole net in bf16 (fp32 PSUM accumulation); early relus on the vector
    engine; ~40 warm-up matmuls in the ~10 us DMA-prologue dead zone
    release the PE HAM clock gate before the serial early chain.
  - Every PSUM accumulator gets its own (bank-aligned) pool buffer: matmul
    dst slices at sub-bank offsets silently corrupt accumulation (measured).
  - Measured rel err vs the fp32 reference: ~7.5e-3 (gate 2e-2).

kernel(**inputs) takes the FULL unsharded inputs and returns the FULL output.
"""

import ml_dtypes
import numpy as np

NCORES = 8
B = 16
P = 128
C = 8
NLVL = 10
BF16 = ml_dtypes.bfloat16

# stream column offsets (bf16 elems per partition; chunks of 8192)
OFF_W7 = 0
OFF_W8 = 8192           # + 8192*c (co-chunk c)
OFF_W9 = 40960          # + 2048*m
OFF_W10 = 57344         # + 4096*u
NCHUNK = 15

_CACHE = {}


# ---------------------------------------------------------------- host prep

def _host_prep(inputs):
    """Build the per-core input maps (numpy only)."""
    ind = np.ascontiguousarray(np.asarray(inputs["in_data"], np.float32))
    f = {l: np.asarray(inputs[f"f{l}"], np.float32) for l in range(1, NLVL + 1)}
    f0 = np.asarray(inputs["in_filter"], np.float32)     # [2, 1, 8]
    fd = np.asarray(inputs["fea_dense"], np.float32)     # [1024, 8, 2]

    shared = {}
    # c32 [32, 1152] bf16: r0 (1024) | w0 (128)
    r0 = ind[:, :, 0].reshape(B, 64, 32).transpose(2, 1, 0)   # [32, 64, 16]
    w0 = np.zeros((32, 128), np.float32)
    for wsub in range(16):
        for k in range(2):
            w0[2 * wsub + k, wsub * 8:wsub * 8 + 8] = f0[k, 0, :]
    shared["c32"] = np.ascontiguousarray(np.concatenate(
        [r0.reshape(32, 1024), w0], axis=1).astype(BF16))

    # c128 [128, 3072] bf16: wpk (512) | w5 (512) | w6 (2048)
    wpk = np.zeros((4, 128, 128), np.float32)
    for lvl in range(1, 5):
        cin = 2 ** (lvl - 1) * C
        cout = 2 ** lvl * C
        s_out = (128 // cin) // 2
        for wso in range(s_out):
            for k in range(2):
                wsi = 2 * wso + k
                wpk[lvl - 1, wsi * cin:(wsi + 1) * cin,
                    wso * cout:(wso + 1) * cout] = f[lvl][k]
    w5p = f[5].astype(BF16).reshape(2, 1, 128, 256).transpose(2, 0, 1, 3).reshape(128, 512)
    w6p = f[6].astype(BF16).reshape(2, 2, 128, 512).transpose(2, 0, 1, 3).reshape(128, 2048)
    shared["c128"] = np.ascontiguousarray(np.concatenate(
        [wpk.transpose(1, 0, 2).reshape(128, 512).astype(BF16), w5p, w6p], axis=1))

    w7p = f[7].astype(BF16).reshape(2, 4, 128, 1024).transpose(2, 0, 1, 3).reshape(128, 8192)
    f8b = f[8].astype(BF16)
    f9b = f[9].astype(BF16)
    f10b = f[10].astype(BF16)
    # w8 replicated: 4 co-chunks [128, kt=16, 512] (one 2 MiB chunk each)
    w8cks = [f8b[:, :, c * 512:(c + 1) * 512]
             .reshape(2, 8, 128, 512).transpose(2, 0, 1, 3).reshape(128, 8192)
             for c in range(4)]

    in_maps = []
    for r in range(NCORES):
        # w9 output shard (512 couts): units m = k*4+q of [128, (j c)]
        blk9 = f9b[:, :, r * 512:(r + 1) * 512]
        v9 = blk9.reshape(2, 4, 4, 128, 512).transpose(0, 1, 3, 2, 4)
        w9u = [v9[mm // 4, mm % 4].reshape(128, 2048) for mm in range(8)]
        # w10 output shard (1024 couts): units u = k*8+q of [128, (j c)]
        blk10 = f10b[:, :, r * 1024:(r + 1) * 1024]
        v10 = blk10.reshape(2, 8, 4, 128, 1024).transpose(0, 1, 3, 2, 4)
        w10u = [v10[u // 8, u % 8].reshape(128, 4096) for u in range(16)]

        flat = np.concatenate([w7p] + w8cks + w9u + w10u, axis=1)
        ws = np.ascontiguousarray(
            flat.reshape(128, NCHUNK, 8192).transpose(1, 0, 2))

        blkfd = fd[r * 128:(r + 1) * 128]
        fdt = np.ascontiguousarray(np.broadcast_to(
            blkfd.transpose(2, 0, 1).reshape(2, 1024)[None], (B, 2, 1024)))

        m = dict(shared)
        m["ws"] = ws
        m["fdt"] = fdt
        in_maps.append(m)
    return in_maps


# ---------------------------------------------------------------- bass build

def _build():
    import concourse.bass as bass
    import concourse.mybir as mybir
    import concourse.tile as tile
    from concourse import bacc

    f32 = mybir.dt.float32
    bf16 = mybir.dt.bfloat16
    RELU = mybir.ActivationFunctionType.Relu

    nc = bacc.Bacc("TRN2", target_bir_lowering=False, debug=False,
                   num_devices=NCORES)

    def inp(name, shape, dt):
        return nc.dram_tensor(name, shape, dt, kind="ExternalInput").ap()

    c32 = inp("c32", [32, 1152], bf16)
    c128 = inp("c128", [128, 3072], bf16)
    ws = inp("ws", [NCHUNK, 128, 8192], bf16)
    fdt = inp("fdt", [B, 2, 1024], f32)
    out = nc.dram_tensor("out", [B, 128, 2], f32, kind="ExternalOutput").ap()

    def chunk_slice(ck, col, n):
        c, off = divmod(col, 8192)
        assert off + n <= 8192, (col, n)
        return ck[c][:, off:off + n]

    with tile.TileContext(nc) as tc:
        with (
            tc.tile_pool(name="const", bufs=1) as constp,
            tc.tile_pool(name="actp", bufs=3) as actp,
            tc.tile_pool(name="wsp", bufs=9) as wsp,
            tc.tile_pool(name="bigp", bufs=1) as bigp,
            tc.tile_pool(name="psA", bufs=2, space="PSUM") as psA,
            tc.tile_pool(name="psB", bufs=4, space="PSUM") as psB,
            tc.tile_pool(name="psC", bufs=2, space="PSUM") as psC,
            tc.tile_pool(name="dramp", bufs=1, space="DRAM") as dramp,
        ):
            ag9_in = dramp.tile([1, 128, 4, 2, 16], bf16, name="ag9_in")
            ag9_out = dramp.tile([NCORES, 128, 4, 2, 16], bf16, name="ag9_out",
                                 addr_space="Shared")

            # ---- PE warm-up in the DMA-prologue dead zone (HAM clock gate)
            wrm = bigp.tile([128, 512], bf16, name="wrm")
            nc.vector.memset(wrm[:], 0)
            wps = psC.tile([B, 512], f32, name="wps", tag="psC")
            for _ in range(40):
                nc.tensor.matmul(wps[:], wrm[:, 0:16], wrm[:],
                                 start=True, stop=True)

            # ---- sync ring FIFO: consts then the weight stream
            c32sb = constp.tile([32, 1152], bf16, name="c32sb")
            nc.sync.dma_start(c32sb[:], c32)
            c128sb = constp.tile([128, 3072], bf16, name="c128sb")
            nc.sync.dma_start(c128sb[:], c128)

            x9loc = bigp.tile([128, 4, 2, 16], bf16, name="x9loc")
            x9g = bigp.tile([128, NCORES, 4, 2, 16], bf16, name="x9g")

            ck = []

            def stream_chunks(upto):
                while len(ck) < upto:
                    i = len(ck)
                    t = wsp.tile([128, 8192], bf16, name=f"ck{i}", tag="ws")
                    nc.sync.dma_start(t[:], ws[i])
                    ck.append(t)

            stream_chunks(8)    # w7, w8 x4, w9 m0..m5, w10 u0,u1 in flight

            # ---- scalar ring: only end-of-kernel traffic
            fdsb = constp.tile([B, 2, 1024], f32, name="fdsb")
            nc.scalar.dma_start(fdsb[:], fdt)

            r0v = c32sb[:, 0:1024].rearrange("p (w b) -> p w b", b=16)
            w0v = c32sb[:, 1024:1152]
            wpkv = c128sb[:, 0:512].rearrange("p (l c) -> p l c", c=128)

            # ---- input conv + packed levels 1..4 (bf16, vector relu)
            xprev = None
            for lvl in range(5):
                xn = actp.tile([128, 64, 16], bf16, name=f"x{lvl}", tag="xl")
                for ch in range(2):
                    ps = psA.tile([128, 32, 16], f32, name="psA", tag="psA")
                    if lvl == 0:
                        nc.tensor.matmul(
                            ps[:], w0v, r0v[:, ch * 32:(ch + 1) * 32, :],
                            start=True, stop=True)
                    else:
                        nc.tensor.matmul(
                            ps[:], wpkv[:, lvl - 1],
                            xprev[:, ch * 32:(ch + 1) * 32, :],
                            start=True, stop=True)
                    nc.vector.tensor_scalar_max(
                        xn[:, ch * 32:(ch + 1) * 32, :], ps[:], 0.0)
                xprev = xn

            # ---- standard levels 5..7 (weights stationary, vector relu)
            def std_level(xin, wsb, cin_t, cout_t, w_out, name):
                xn = actp.tile([128, cout_t, w_out, 16], bf16,
                               name=name, tag="xl")
                for ct in range(cout_t):
                    ps = psA.tile([128, w_out, 16], f32, name="psA", tag="psA")
                    for cit in range(cin_t):
                        rhs2 = xin[:, cit].rearrange(
                            "p (w two) b -> p two w b", two=2)
                        for k in range(2):
                            nc.tensor.matmul(
                                ps[:],
                                wsb[:, k * cin_t + cit,
                                    ct * 128:(ct + 1) * 128],
                                rhs2[:, k],
                                start=(cit == 0 and k == 0),
                                stop=(cit == cin_t - 1 and k == 1))
                    nc.vector.tensor_scalar_max(xn[:, ct], ps[:], 0.0)
                return xn

            w5v = c128sb[:, 512:1024].rearrange("p (t c) -> p t c", c=256)
            w6v = c128sb[:, 1024:3072].rearrange("p (t c) -> p t c", c=512)
            w7v = chunk_slice(ck, OFF_W7, 8192).rearrange(
                "p (t c) -> p t c", c=1024)
            x5 = std_level(xprev[:, None], w5v, 1, 2, 32, "x5")
            x6 = std_level(x5, w6v, 2, 4, 16, "x6")
            x7 = std_level(x6, w7v, 4, 8, 8, "x7")

            # ---- level 8 REPLICATED (2048 couts), consuming ck1..ck4
            x8sb = bigp.tile([128, 16, 4, 16], bf16, name="x8sb")
            for c in range(4):
                w8v = chunk_slice(ck, OFF_W8 + 8192 * c, 8192).rearrange(
                    "p (t c) -> p t c", c=512)
                for ctl in range(4):
                    ps = psA.tile([128, 4, 16], f32, name="psA", tag="psA")
                    for cit in range(8):
                        rhs2 = x7[:, cit].rearrange(
                            "p (w two) b -> p two w b", two=2)
                        for k in range(2):
                            nc.tensor.matmul(
                                ps[:],
                                w8v[:, k * 8 + cit,
                                    ctl * 128:(ctl + 1) * 128],
                                rhs2[:, k],
                                start=(cit == 0 and k == 0),
                                stop=(cit == 7 and k == 1))
                    nc.vector.tensor_scalar_max(x8sb[:, c * 4 + ctl], ps[:], 0.0)

            # ---- level 9 (512-cout shard; 4 bank-aligned accumulators)
            ps9 = [psB.tile([128, 2, 16], f32, name=f"ps9_{ct}", tag="psB")
                   for ct in range(4)]
            for m in range(8):
                k, q = divmod(m, 4)
                for j in range(4):
                    cit = q * 4 + j
                    rhs = x8sb[:, cit].rearrange(
                        "p (w two) b -> p two w b", two=2)[:, k]
                    for ct in range(4):
                        nc.tensor.matmul(
                            ps9[ct][:],
                            chunk_slice(ck, OFF_W9 + 2048 * m + 512 * j
                                        + 128 * ct, 128),
                            rhs,
                            start=(m == 0 and j == 0),
                            stop=(m == 7 and j == 3))
            for ct in range(4):
                nc.scalar.activation(x9loc[:, ct], ps9[ct][:], RELU)

            # ---- the single AllGather (x9). Bounce store rides the sync
            # FIFO after ck7 (~57 us); the gather load after ck12 (~85 us)
            # pauses the stream exactly at the ncfw readiness floor so the
            # data phase runs uncontended.
            nc.sync.dma_start(ag9_in[0], x9loc[:])
            nc.gpsimd.collective_compute(
                "AllGather", mybir.AluOpType.bypass,
                replica_groups=[list(range(NCORES))],
                ins=[ag9_in.opt()], outs=[ag9_out.opt()])
            stream_chunks(13)
            nc.sync.dma_start(
                x9g[:], ag9_out[:].rearrange("r p t k b -> p r t k b"))
            stream_chunks(NCHUNK)

            # ---- level 10 (1024-cout shard, acts stationary)
            ps10 = [psC.tile([B, 512], f32, name=f"ps10_{cb}", tag="psC")
                    for cb in range(2)]
            for u in range(16):
                k, q = divmod(u, 8)
                for j in range(4):
                    lhsT = x9g[:, q, j, k, :]
                    for cb in range(2):
                        nc.tensor.matmul(
                            ps10[cb][:], lhsT,
                            chunk_slice(ck, OFF_W10 + 4096 * u + 1024 * j
                                        + 512 * cb, 512),
                            start=(u == 0 and j == 0),
                            stop=(u == 15 and j == 3))
            x10 = bigp.tile([B, 1024], f32, name="x10")
            for cb in range(2):
                nc.scalar.activation(
                    x10[:, cb * 512:(cb + 1) * 512], ps10[cb][:], RELU)

            # ---- final per-block einsum on the vector engine
            osb = bigp.tile([B, 128, 2], f32, name="osb")
            for o in range(2):
                prod = bigp.tile([B, 1024], f32, name=f"prod{o}")
                nc.vector.tensor_tensor(
                    prod[:], x10[:], fdsb[:, o, :], mybir.AluOpType.mult)
                nc.vector.tensor_reduce(
                    osb[:, :, o],
                    prod.rearrange("p (k c) -> p k c", c=8),
                    mybir.AxisListType.X, mybir.AluOpType.add)
            nc.scalar.dma_start(out, osb[:])

    nc.compile()
    return nc


# ------------------------------------------------------------------- kernel

def kernel(**inputs):
    from concourse.bass_utils import run_bass_kernel_spmd

    in_maps = _host_prep(inputs)
    if "nc" not in _CACHE:
        _CACHE["nc"] = _build()
    nc = _CACHE["nc"]
    res = run_bass_kernel_spmd(nc, in_maps, core_ids=list(range(NCORES)))
    parts = [res.results[r]["out"] for r in range(NCORES)]  # each [16, 128, 2]
    full = np.concatenate(parts, axis=1)                    # [16, 1024, 2]
    return np.ascontiguousarray(full.reshape(B, 2048, 1).astype(np.float32))
